# revision 10
# baseline (speedup 1.0000x reference)
"""BackwardDense (linear branch) on 8 TRN2 NeuronCores.

Math: the pos/neg split in the reference is linear (pos + neg == original), so

  w_out_u_ = einsum('io,bkon->bin', kernel, w_out_u)[:, None]    # [B,1,DIN,N]
  b_out_u_ = b_out_u + einsum('bkon,o->bkn', w_out_u, bias)      # [B,K,N]
  (same for *_l)

Per batch b this is one matmul: out[b] = Aᵀ @ w[b,0], where
A = concat([kernel, bias[None]], 0) is [DIN+1, DOUT] = [65, 512].
Row 64 of the product is the bias-reduction row; rows 0..63 are w_out_*_.

Sharding: data-parallel over B across 8 cores (4 batches/core, u and l both
local, no cross-core communication). Each core:
  - streams w chunks [128, 2048] fp32 from DRAM (32 x 1 MiB DMAs),
  - accumulates PSUM [65, 2048] over the 4 contraction chunks,
  - copies w-rows of u/l into one packed [128, 2048] SBUF tile (full-width
    output DMA), adds b_out to row 64 on partition 64 (partition-aligned),
  - writes ow [4,2,64,2048] and ob [4,2,2048] back to DRAM.
"""

import numpy as np

B, DIN, DOUT, N = 32, 64, 512, 2048
NCORES = 8
BL = B // NCORES  # local batches per core
KC = DOUT // 128  # contraction chunks

_cache = {}


def _build_nc():
    import concourse.bacc as bacc
    import concourse.bass as bass
    import concourse.mybir as mybir
    import concourse.tile as tile

    nc = bacc.Bacc(
        "TRN2",
        target_bir_lowering=False,
        debug=False,
        enable_asserts=False,
        num_devices=NCORES,
    )

    f32 = mybir.dt.float32
    wu = nc.dram_tensor("wu", [BL, KC, 128, N], f32, kind="ExternalInput")
    wl = nc.dram_tensor("wl", [BL, KC, 128, N], f32, kind="ExternalInput")
    ka = nc.dram_tensor("ka", [128, KC, 65], f32, kind="ExternalInput")
    bb = nc.dram_tensor("bb", [BL, 2, N], f32, kind="ExternalInput")
    ow = nc.dram_tensor("ow", [BL, 2, 64, N], f32, kind="ExternalOutput")
    ob = nc.dram_tensor("ob", [BL, 2, N], f32, kind="ExternalOutput")

    with tile.TileContext(nc) as tc:
        with (
            tc.tile_pool(name="const", bufs=1) as cpool,
            tc.tile_pool(name="w", bufs=3) as wpool,
            tc.tile_pool(name="ot", bufs=3) as opool,
            tc.tile_pool(name="obt", bufs=2) as obpool,
            tc.tile_pool(name="psum", bufs=2, space=bass.MemorySpace.PSUM) as pspool,
        ):
            kat = cpool.tile([128, KC, 65], f32)
            # PE warm-up: HAM starts cold (1.2 GHz) and needs ~3.4 us of
            # sustained activity to un-throttle. Run dummy matmuls on zeroed
            # scratch while the first DMAs land so the real stream is warm.
            scratch = cpool.tile([128, 512], f32)
            nc.vector.memset(scratch[:], 0.0)

            first = True
            for b in range(BL):
                obt = obpool.tile([65, 2, N], f32, tag="obt")
                ot = opool.tile([128, N], f32, tag="ot")
                for t, w_in in enumerate((wu, wl)):
                    ps = pspool.tile([65, N], f32, tag="ps")
                    wtk = wpool.tile([128, KC, N], f32, tag="wtk")
                    if first:
                        # Get PE going ASAP: tiny first transfer, then the
                        # constants, then the rest of the first block.
                        nc.sync.dma_start(wtk[:, 0, 0:512], w_in[b, 0, :, 0:512])
                        nc.sync.dma_start(kat[:], ka[:])
                        nc.sync.dma_start(wtk[:, 0, 512:N], w_in[b, 0, :, 512:N])
                        for k in range(1, KC):
                            nc.sync.dma_start(wtk[:, k, :], w_in[b, k])
                    else:
                        for j in range(KC // 2):
                            # 2 contraction chunks per DMA (2 MiB)
                            nc.sync.dma_start(
                                wtk[:, 2 * j : 2 * j + 2, :],
                                w_in[b, 2 * j : 2 * j + 2].rearrange("k p n -> p k n"),
                            )
                    if t == 0:
                        nc.scalar.dma_start(obt[64:65, :, :], bb[b][None])
                    if first:
                        for _ in range(8):
                            nc.tensor.matmul(
                                ps[:, 0:512],
                                scratch[:, 0:65],
                                scratch[:],
                                start=True,
                                stop=True,
                                skip_group_check=True,
                            )
                    for k in range(KC):
                        for n in range(N // 512):
                            nc.tensor.matmul(
                                ps[:, n * 512 : (n + 1) * 512],
                                kat[:, k, :],
                                wtk[:, k, n * 512 : (n + 1) * 512],
                                start=(k == 0),
                                stop=(k == KC - 1),
                            )
                    first = False
                    # w rows: u -> partitions 0..63, l -> partitions 64..127.
                    # Per-bank copies so PSUM banks free as soon as their last
                    # accumulation lands.
                    for n in range(N // 512):
                        nc.vector.tensor_copy(
                            ot[t * 64 : (t + 1) * 64, n * 512 : (n + 1) * 512],
                            ps[0:64, n * 512 : (n + 1) * 512],
                        )
                        # bias row (partition 64, aligned): b_out + reduction row
                        nc.vector.tensor_add(
                            obt[64:65, t, n * 512 : (n + 1) * 512],
                            ps[64:65, n * 512 : (n + 1) * 512],
                            obt[64:65, t, n * 512 : (n + 1) * 512],
                        )
                owb = ow[b].rearrange("t m n -> (t m) n")
                for n in range(N // 512):
                    nc.sync.dma_start(
                        owb[:, n * 512 : (n + 1) * 512], ot[:, n * 512 : (n + 1) * 512]
                    )
                nc.scalar.dma_start(ob[b][None], obt[64:65, :, :])

    nc.compile()
    return nc


def _get_nc():
    if "nc" not in _cache:
        _cache["nc"] = _build_nc()
    return _cache["nc"]


def _prep_in_maps(kern, bias, w_out_u, b_out_u, w_out_l, b_out_l):
    kaug = np.concatenate([kern, bias[None, :]], axis=0)  # [65, 512]
    ka = np.ascontiguousarray(
        kaug.T.reshape(KC, 128, 65).transpose(1, 0, 2)
    )  # [128, KC, 65]; ka[p,k,m] = kaug[m, k*128+p]

    in_maps = []
    for c in range(NCORES):
        s = slice(c * BL, (c + 1) * BL)
        in_maps.append(
            {
                "wu": w_out_u[s, 0].reshape(BL, KC, 128, N),
                "wl": w_out_l[s, 0].reshape(BL, KC, 128, N),
                "ka": ka,
                "bb": np.ascontiguousarray(
                    np.stack([b_out_u[s, 0], b_out_l[s, 0]], axis=1)
                ),
            }
        )
    return in_maps


def _assemble(results):
    ow = np.concatenate([r["ow"] for r in results], axis=0)  # [B, 2, 64, N]
    ob = np.concatenate([r["ob"] for r in results], axis=0)  # [B, 2, N]
    w_u = np.ascontiguousarray(ow[:, None, 0])  # [B, 1, 64, N]
    w_l = np.ascontiguousarray(ow[:, None, 1])
    b_u = np.ascontiguousarray(ob[:, None, 0])  # [B, 1, N]
    b_l = np.ascontiguousarray(ob[:, None, 1])
    return (w_u, b_u, w_l, b_l)


def kernel(**inputs):
    from concourse.bass_utils import run_bass_kernel_spmd

    kern = np.ascontiguousarray(np.asarray(inputs["kernel"], dtype=np.float32))
    bias = np.ascontiguousarray(np.asarray(inputs["bias"], dtype=np.float32))
    w_out_u = np.ascontiguousarray(np.asarray(inputs["w_out_u"], dtype=np.float32))
    b_out_u = np.ascontiguousarray(np.asarray(inputs["b_out_u"], dtype=np.float32))
    w_out_l = np.ascontiguousarray(np.asarray(inputs["w_out_l"], dtype=np.float32))
    b_out_l = np.ascontiguousarray(np.asarray(inputs["b_out_l"], dtype=np.float32))

    nc = _get_nc()
    in_maps = _prep_in_maps(kern, bias, w_out_u, b_out_u, w_out_l, b_out_l)
    res = run_bass_kernel_spmd(nc, in_maps, core_ids=list(range(NCORES)))
    return _assemble(res.results)


# revision 11
# speedup vs baseline: 1.0072x; 1.0072x over previous
"""BackwardDense (linear branch) on 8 TRN2 NeuronCores.

Math: the pos/neg split in the reference is linear (pos + neg == original), so

  w_out_u_ = einsum('io,bkon->bin', kernel, w_out_u)[:, None]    # [B,1,DIN,N]
  b_out_u_ = b_out_u + einsum('bkon,o->bkn', w_out_u, bias)      # [B,K,N]
  (same for *_l)

Per batch b this is one matmul: out[b] = Aᵀ @ w[b,0], where
A = concat([kernel, bias[None]], 0) is [DIN+1, DOUT] = [65, 512].
Row 64 of the product is the bias-reduction row; rows 0..63 are w_out_*_.

Sharding: data-parallel over B across 8 cores (4 batches/core, u and l both
local, no cross-core communication). Each core:
  - streams w chunks [128, 2048] fp32 from DRAM (32 x 1 MiB DMAs),
  - accumulates PSUM [65, 2048] over the 4 contraction chunks,
  - copies w-rows of u/l into one packed [128, 2048] SBUF tile (full-width
    output DMA), adds b_out to row 64 on partition 64 (partition-aligned),
  - writes ow [4,2,64,2048] and ob [4,2,2048] back to DRAM.
"""

import numpy as np

B, DIN, DOUT, N = 32, 64, 512, 2048
NCORES = 8
BL = B // NCORES  # local batches per core
KC = DOUT // 128  # contraction chunks

_cache = {}


def _build_nc():
    import concourse.bacc as bacc
    import concourse.bass as bass
    import concourse.mybir as mybir
    import concourse.tile as tile

    nc = bacc.Bacc(
        "TRN2",
        target_bir_lowering=False,
        debug=False,
        enable_asserts=False,
        num_devices=NCORES,
    )

    f32 = mybir.dt.float32
    wu = nc.dram_tensor("wu", [BL, KC, 128, N], f32, kind="ExternalInput")
    wl = nc.dram_tensor("wl", [BL, KC, 128, N], f32, kind="ExternalInput")
    ka = nc.dram_tensor("ka", [128, KC, 65], f32, kind="ExternalInput")
    bb = nc.dram_tensor("bb", [BL, 2, N], f32, kind="ExternalInput")
    ow = nc.dram_tensor("ow", [BL, 2, 64, N], f32, kind="ExternalOutput")
    ob = nc.dram_tensor("ob", [BL, 2, N], f32, kind="ExternalOutput")

    with tile.TileContext(nc) as tc:
        with (
            tc.tile_pool(name="const", bufs=1) as cpool,
            tc.tile_pool(name="w", bufs=3) as wpool,
            tc.tile_pool(name="ot", bufs=3) as opool,
            tc.tile_pool(name="obt", bufs=2) as obpool,
            tc.tile_pool(name="psum", bufs=2, space=bass.MemorySpace.PSUM) as pspool,
        ):
            kat = cpool.tile([128, KC, 65], f32)
            # PE warm-up: HAM starts cold (1.2 GHz) and needs ~3.4 us of
            # sustained activity to un-throttle. Run dummy matmuls on zeroed
            # scratch while the first DMAs land so the real stream is warm.
            scratch = cpool.tile([128, 512], f32)
            nc.vector.memset(scratch[:], 0.0)

            first = True
            for b in range(BL):
                obt = obpool.tile([65, 2, N], f32, tag="obt")
                ot = opool.tile([128, N], f32, tag="ot")
                for t, w_in in enumerate((wu, wl)):
                    ps = pspool.tile([65, N], f32, tag="ps")
                    wtk = wpool.tile([128, KC, N], f32, tag="wtk")
                    if first:
                        # Get PE going ASAP: tiny first transfer, then the
                        # constants, then the rest of the first block.
                        nc.sync.dma_start(wtk[:, 0, 0:512], w_in[b, 0, :, 0:512])
                        nc.sync.dma_start(kat[:], ka[:])
                        nc.sync.dma_start(wtk[:, 0, 512:N], w_in[b, 0, :, 512:N])
                        for k in range(1, KC):
                            nc.sync.dma_start(wtk[:, k, :], w_in[b, k])
                    else:
                        for j in range(KC // 2):
                            # 2 contraction chunks per DMA (2 MiB)
                            nc.sync.dma_start(
                                wtk[:, 2 * j : 2 * j + 2, :],
                                w_in[b, 2 * j : 2 * j + 2].rearrange("k p n -> p k n"),
                            )
                    if t == 0:
                        nc.scalar.dma_start(obt[64:65, :, :], bb[b][None])
                    if first:
                        for _ in range(8):
                            nc.tensor.matmul(
                                ps[:, 0:512],
                                scratch[:, 0:65],
                                scratch[:],
                                start=True,
                                stop=True,
                                skip_group_check=True,
                            )
                    for k in range(KC):
                        for n in range(N // 512):
                            nc.tensor.matmul(
                                ps[:, n * 512 : (n + 1) * 512],
                                kat[:, k, :],
                                wtk[:, k, n * 512 : (n + 1) * 512],
                                start=(k == 0),
                                stop=(k == KC - 1),
                            )
                    first = False
                    # w rows: u -> partitions 0..63, l -> partitions 64..127.
                    # Per-bank copies so PSUM banks free as soon as their last
                    # accumulation lands.
                    for n in range(N // 512):
                        nc.vector.tensor_copy(
                            ot[t * 64 : (t + 1) * 64, n * 512 : (n + 1) * 512],
                            ps[0:64, n * 512 : (n + 1) * 512],
                        )
                        # bias row (partition 64, aligned): b_out + reduction row
                        nc.vector.tensor_add(
                            obt[64:65, t, n * 512 : (n + 1) * 512],
                            ps[64:65, n * 512 : (n + 1) * 512],
                            obt[64:65, t, n * 512 : (n + 1) * 512],
                        )
                owb = ow[b].rearrange("t m n -> (t m) n")
                nc.sync.dma_start(owb[:, 0 : N // 2], ot[:, 0 : N // 2])
                nc.sync.dma_start(owb[:, N // 2 : N], ot[:, N // 2 : N])
                nc.scalar.dma_start(ob[b][None], obt[64:65, :, :])

    nc.compile()
    return nc


def _get_nc():
    if "nc" not in _cache:
        _cache["nc"] = _build_nc()
    return _cache["nc"]


def _prep_in_maps(kern, bias, w_out_u, b_out_u, w_out_l, b_out_l):
    kaug = np.concatenate([kern, bias[None, :]], axis=0)  # [65, 512]
    ka = np.ascontiguousarray(
        kaug.T.reshape(KC, 128, 65).transpose(1, 0, 2)
    )  # [128, KC, 65]; ka[p,k,m] = kaug[m, k*128+p]

    in_maps = []
    for c in range(NCORES):
        s = slice(c * BL, (c + 1) * BL)
        in_maps.append(
            {
                "wu": w_out_u[s, 0].reshape(BL, KC, 128, N),
                "wl": w_out_l[s, 0].reshape(BL, KC, 128, N),
                "ka": ka,
                "bb": np.ascontiguousarray(
                    np.stack([b_out_u[s, 0], b_out_l[s, 0]], axis=1)
                ),
            }
        )
    return in_maps


def _assemble(results):
    ow = np.concatenate([r["ow"] for r in results], axis=0)  # [B, 2, 64, N]
    ob = np.concatenate([r["ob"] for r in results], axis=0)  # [B, 2, N]
    w_u = np.ascontiguousarray(ow[:, None, 0])  # [B, 1, 64, N]
    w_l = np.ascontiguousarray(ow[:, None, 1])
    b_u = np.ascontiguousarray(ob[:, None, 0])  # [B, 1, N]
    b_l = np.ascontiguousarray(ob[:, None, 1])
    return (w_u, b_u, w_l, b_l)


def kernel(**inputs):
    from concourse.bass_utils import run_bass_kernel_spmd

    kern = np.ascontiguousarray(np.asarray(inputs["kernel"], dtype=np.float32))
    bias = np.ascontiguousarray(np.asarray(inputs["bias"], dtype=np.float32))
    w_out_u = np.ascontiguousarray(np.asarray(inputs["w_out_u"], dtype=np.float32))
    b_out_u = np.ascontiguousarray(np.asarray(inputs["b_out_u"], dtype=np.float32))
    w_out_l = np.ascontiguousarray(np.asarray(inputs["w_out_l"], dtype=np.float32))
    b_out_l = np.ascontiguousarray(np.asarray(inputs["b_out_l"], dtype=np.float32))

    nc = _get_nc()
    in_maps = _prep_in_maps(kern, bias, w_out_u, b_out_u, w_out_l, b_out_l)
    res = run_bass_kernel_spmd(nc, in_maps, core_ids=list(range(NCORES)))
    return _assemble(res.results)


# revision 17
# speedup vs baseline: 1.0194x; 1.0122x over previous
"""BackwardDense (linear branch) on 8 TRN2 NeuronCores.

Math: the pos/neg split in the reference is linear (pos + neg == original), so

  w_out_u_ = einsum('io,bkon->bin', kernel, w_out_u)[:, None]    # [B,1,DIN,N]
  b_out_u_ = b_out_u + einsum('bkon,o->bkn', w_out_u, bias)      # [B,K,N]
  (same for *_l)

Per batch b this is one matmul: out[b] = Aᵀ @ w[b,0], where
A = concat([kernel, bias[None]], 0) is [DIN+1, DOUT] = [65, 512].
Row 64 of the product is the bias-reduction row; rows 0..63 are w_out_*_.

Sharding: data-parallel over B across 8 cores (4 batches/core, u and l both
local, no cross-core communication). Each core:
  - streams w chunks [128, 2048] fp32 from DRAM (32 x 1 MiB DMAs),
  - accumulates PSUM [65, 2048] over the 4 contraction chunks,
  - copies w-rows of u/l into one packed [128, 2048] SBUF tile (full-width
    output DMA), adds b_out to row 64 on partition 64 (partition-aligned),
  - writes ow [4,2,64,2048] and ob [4,2,2048] back to DRAM.
"""

import numpy as np

B, DIN, DOUT, N = 32, 64, 512, 2048
NCORES = 8
BL = B // NCORES  # local batches per core
KC = DOUT // 128  # contraction chunks

_cache = {}


def _build_nc():
    import concourse.bacc as bacc
    import concourse.bass as bass
    import concourse.mybir as mybir
    import concourse.tile as tile

    nc = bacc.Bacc(
        "TRN2",
        target_bir_lowering=False,
        debug=False,
        enable_asserts=False,
        num_devices=NCORES,
    )

    f32 = mybir.dt.float32
    wu = nc.dram_tensor("wu", [BL, KC, 128, N], f32, kind="ExternalInput")
    wl = nc.dram_tensor("wl", [BL, KC, 128, N], f32, kind="ExternalInput")
    ka = nc.dram_tensor("ka", [128, KC, 65], f32, kind="ExternalInput")
    bb = nc.dram_tensor("bb", [BL, 2, N], f32, kind="ExternalInput")
    ow = nc.dram_tensor("ow", [BL, 2, 64, N], f32, kind="ExternalOutput")
    ob = nc.dram_tensor("ob", [BL, 2, N], f32, kind="ExternalOutput")

    with tile.TileContext(nc) as tc:
        with (
            tc.tile_pool(name="const", bufs=1) as cpool,
            tc.tile_pool(name="w", bufs=4) as wpool,
            tc.tile_pool(name="ot", bufs=4) as opool,
            tc.tile_pool(name="obt", bufs=2) as obpool,
            tc.tile_pool(name="psum", bufs=2, space=bass.MemorySpace.PSUM) as pspool,
        ):
            kat = cpool.tile([128, KC, 65], f32)
            # PE warm-up: HAM starts cold (1.2 GHz) and needs ~3.4 us of
            # sustained activity to un-throttle. Run dummy matmuls on zeroed
            # scratch while the first DMAs land so the real stream is warm.
            scratch = cpool.tile([128, 512], f32)
            nc.vector.memset(scratch[:], 0.0)

            first = True
            for b in range(BL):
                obt = obpool.tile([65, 2, N], f32, tag="obt")
                ot = opool.tile([128, N], f32, tag="ot")
                for t, w_in in enumerate((wu, wl)):
                    ps = pspool.tile([65, N], f32, tag="ps")
                    wtk = wpool.tile([128, KC, N], f32, tag="wtk")
                    if first:
                        # Get PE going ASAP: tiny first transfer, then the
                        # constants, then the rest of the first block.
                        nc.sync.dma_start(wtk[:, 0, 0:512], w_in[b, 0, :, 0:512])
                        nc.sync.dma_start(kat[:], ka[:])
                        nc.sync.dma_start(wtk[:, 0, 512:N], w_in[b, 0, :, 512:N])
                        for k in range(1, KC):
                            nc.sync.dma_start(wtk[:, k, :], w_in[b, k])
                    else:
                        for j in range(KC // 2):
                            # 2 contraction chunks per DMA (2 MiB)
                            nc.sync.dma_start(
                                wtk[:, 2 * j : 2 * j + 2, :],
                                w_in[b, 2 * j : 2 * j + 2].rearrange("k p n -> p k n"),
                            )
                    if t == 0:
                        nc.scalar.dma_start(obt[64:65, :, :], bb[b][None])
                    if first:
                        for _ in range(6):
                            nc.tensor.matmul(
                                ps[:, 0:512],
                                scratch[:, 0:65],
                                scratch[:],
                                start=True,
                                stop=True,
                                skip_group_check=True,
                            )
                    for k in range(KC):
                        for n in range(N // 512):
                            nc.tensor.matmul(
                                ps[:, n * 512 : (n + 1) * 512],
                                kat[:, k, :],
                                wtk[:, k, n * 512 : (n + 1) * 512],
                                start=(k == 0),
                                stop=(k == KC - 1),
                            )
                    first = False
                    # w rows: u -> partitions 0..63, l -> partitions 64..127.
                    # Per-bank copies so PSUM banks free as soon as their last
                    # accumulation lands.
                    for n in range(N // 512):
                        nc.vector.tensor_copy(
                            ot[t * 64 : (t + 1) * 64, n * 512 : (n + 1) * 512],
                            ps[0:64, n * 512 : (n + 1) * 512],
                        )
                        # bias row (partition 64, aligned): b_out + reduction row
                        nc.vector.tensor_add(
                            obt[64:65, t, n * 512 : (n + 1) * 512],
                            ps[64:65, n * 512 : (n + 1) * 512],
                            obt[64:65, t, n * 512 : (n + 1) * 512],
                        )
                owb = ow[b].rearrange("t m n -> (t m) n")
                nc.sync.dma_start(owb[:, 0 : N // 2], ot[:, 0 : N // 2])
                nc.sync.dma_start(owb[:, N // 2 : N], ot[:, N // 2 : N])
                nc.scalar.dma_start(ob[b][None], obt[64:65, :, :])

    nc.compile()
    return nc


def _get_nc():
    if "nc" not in _cache:
        _cache["nc"] = _build_nc()
    return _cache["nc"]


def _prep_in_maps(kern, bias, w_out_u, b_out_u, w_out_l, b_out_l):
    kaug = np.concatenate([kern, bias[None, :]], axis=0)  # [65, 512]
    ka = np.ascontiguousarray(
        kaug.T.reshape(KC, 128, 65).transpose(1, 0, 2)
    )  # [128, KC, 65]; ka[p,k,m] = kaug[m, k*128+p]

    in_maps = []
    for c in range(NCORES):
        s = slice(c * BL, (c + 1) * BL)
        in_maps.append(
            {
                "wu": w_out_u[s, 0].reshape(BL, KC, 128, N),
                "wl": w_out_l[s, 0].reshape(BL, KC, 128, N),
                "ka": ka,
                "bb": np.ascontiguousarray(
                    np.stack([b_out_u[s, 0], b_out_l[s, 0]], axis=1)
                ),
            }
        )
    return in_maps


def _assemble(results):
    ow = np.concatenate([r["ow"] for r in results], axis=0)  # [B, 2, 64, N]
    ob = np.concatenate([r["ob"] for r in results], axis=0)  # [B, 2, N]
    w_u = np.ascontiguousarray(ow[:, None, 0])  # [B, 1, 64, N]
    w_l = np.ascontiguousarray(ow[:, None, 1])
    b_u = np.ascontiguousarray(ob[:, None, 0])  # [B, 1, N]
    b_l = np.ascontiguousarray(ob[:, None, 1])
    return (w_u, b_u, w_l, b_l)


def kernel(**inputs):
    from concourse.bass_utils import run_bass_kernel_spmd

    kern = np.ascontiguousarray(np.asarray(inputs["kernel"], dtype=np.float32))
    bias = np.ascontiguousarray(np.asarray(inputs["bias"], dtype=np.float32))
    w_out_u = np.ascontiguousarray(np.asarray(inputs["w_out_u"], dtype=np.float32))
    b_out_u = np.ascontiguousarray(np.asarray(inputs["b_out_u"], dtype=np.float32))
    w_out_l = np.ascontiguousarray(np.asarray(inputs["w_out_l"], dtype=np.float32))
    b_out_l = np.ascontiguousarray(np.asarray(inputs["b_out_l"], dtype=np.float32))

    nc = _get_nc()
    in_maps = _prep_in_maps(kern, bias, w_out_u, b_out_u, w_out_l, b_out_l)
    res = run_bass_kernel_spmd(nc, in_maps, core_ids=list(range(NCORES)))
    return _assemble(res.results)


# revision 18
# speedup vs baseline: 1.0382x; 1.0184x over previous
"""BackwardDense (linear branch) on 8 TRN2 NeuronCores.

Math: the pos/neg split in the reference is linear (pos + neg == original), so

  w_out_u_ = einsum('io,bkon->bin', kernel, w_out_u)[:, None]    # [B,1,DIN,N]
  b_out_u_ = b_out_u + einsum('bkon,o->bkn', w_out_u, bias)      # [B,K,N]
  (same for *_l)

Per batch b this is one matmul: out[b] = Aᵀ @ w[b,0], where
A = concat([kernel, bias[None]], 0) is [DIN+1, DOUT] = [65, 512].
Row 64 of the product is the bias-reduction row; rows 0..63 are w_out_*_.

Sharding: data-parallel over B across 8 cores (4 batches/core, u and l both
local, no cross-core communication). Each core:
  - streams w chunks [128, 2048] fp32 from DRAM (32 x 1 MiB DMAs),
  - accumulates PSUM [65, 2048] over the 4 contraction chunks,
  - copies w-rows of u/l into one packed [128, 2048] SBUF tile (full-width
    output DMA), adds b_out to row 64 on partition 64 (partition-aligned),
  - writes ow [4,2,64,2048] and ob [4,2,2048] back to DRAM.
"""

import numpy as np

B, DIN, DOUT, N = 32, 64, 512, 2048
NCORES = 8
BL = B // NCORES  # local batches per core
KC = DOUT // 128  # contraction chunks

_cache = {}


def _build_nc():
    import concourse.bacc as bacc
    import concourse.bass as bass
    import concourse.mybir as mybir
    import concourse.tile as tile

    nc = bacc.Bacc(
        "TRN2",
        target_bir_lowering=False,
        debug=False,
        enable_asserts=False,
        num_devices=NCORES,
    )

    f32 = mybir.dt.float32
    wu = nc.dram_tensor("wu", [BL, KC, 128, N], f32, kind="ExternalInput")
    wl = nc.dram_tensor("wl", [BL, KC, 128, N], f32, kind="ExternalInput")
    ka = nc.dram_tensor("ka", [128, KC, 65], f32, kind="ExternalInput")
    bb = nc.dram_tensor("bb", [BL, 2, N], f32, kind="ExternalInput")
    ow = nc.dram_tensor("ow", [BL, 2, 64, N], f32, kind="ExternalOutput")
    ob = nc.dram_tensor("ob", [BL, 2, N], f32, kind="ExternalOutput")

    with tile.TileContext(nc) as tc:
        with (
            tc.tile_pool(name="const", bufs=1) as cpool,
            tc.tile_pool(name="w", bufs=4) as wpool,
            tc.tile_pool(name="ot", bufs=4) as opool,
            tc.tile_pool(name="obt", bufs=2) as obpool,
            tc.tile_pool(name="psum", bufs=2, space=bass.MemorySpace.PSUM) as pspool,
        ):
            kat = cpool.tile([128, KC, 65], f32)
            # PE warm-up: HAM starts cold (1.2 GHz) and needs ~3.4 us of
            # sustained activity to un-throttle. Run dummy matmuls on zeroed
            # scratch while the first DMAs land so the real stream is warm.
            scratch = cpool.tile([128, 512], f32)
            nc.vector.memset(scratch[:], 0.0)

            first = True
            for b in range(BL):
                obt = obpool.tile([65, 2, N], f32, tag="obt")
                ot = opool.tile([128, N], f32, tag="ot")
                for t, w_in in enumerate((wu, wl)):
                    ps = pspool.tile([65, N], f32, tag="ps")
                    wtk = wpool.tile([128, KC, N], f32, tag="wtk")
                    if first:
                        # Get PE going ASAP: tiny first transfer, then the
                        # constants, then the rest of the first block.
                        nc.sync.dma_start(wtk[:, 0, 0:512], w_in[b, 0, :, 0:512])
                        nc.scalar.dma_start(kat[:], ka[:])
                        nc.sync.dma_start(wtk[:, 0, 512:N], w_in[b, 0, :, 512:N])
                        for k in range(1, KC):
                            nc.sync.dma_start(wtk[:, k, :], w_in[b, k])
                    else:
                        for j in range(KC // 2):
                            # 2 contraction chunks per DMA (2 MiB)
                            nc.sync.dma_start(
                                wtk[:, 2 * j : 2 * j + 2, :],
                                w_in[b, 2 * j : 2 * j + 2].rearrange("k p n -> p k n"),
                            )
                    if t == 0:
                        nc.scalar.dma_start(obt[64:65, :, :], bb[b][None])
                    if first:
                        for _ in range(6):
                            nc.tensor.matmul(
                                ps[:, 0:512],
                                scratch[:, 0:65],
                                scratch[:],
                                start=True,
                                stop=True,
                                skip_group_check=True,
                            )
                    for k in range(KC):
                        for n in range(N // 512):
                            nc.tensor.matmul(
                                ps[:, n * 512 : (n + 1) * 512],
                                kat[:, k, :],
                                wtk[:, k, n * 512 : (n + 1) * 512],
                                start=(k == 0),
                                stop=(k == KC - 1),
                            )
                    first = False
                    # w rows: u -> partitions 0..63, l -> partitions 64..127.
                    # Per-bank copies so PSUM banks free as soon as their last
                    # accumulation lands.
                    for n in range(N // 512):
                        nc.vector.tensor_copy(
                            ot[t * 64 : (t + 1) * 64, n * 512 : (n + 1) * 512],
                            ps[0:64, n * 512 : (n + 1) * 512],
                        )
                        # bias row (partition 64, aligned): b_out + reduction row
                        nc.vector.tensor_add(
                            obt[64:65, t, n * 512 : (n + 1) * 512],
                            ps[64:65, n * 512 : (n + 1) * 512],
                            obt[64:65, t, n * 512 : (n + 1) * 512],
                        )
                owb = ow[b].rearrange("t m n -> (t m) n")
                nc.sync.dma_start(owb[:, 0 : N // 2], ot[:, 0 : N // 2])
                nc.sync.dma_start(owb[:, N // 2 : N], ot[:, N // 2 : N])
                nc.scalar.dma_start(ob[b][None], obt[64:65, :, :])

    nc.compile()
    return nc


def _get_nc():
    if "nc" not in _cache:
        _cache["nc"] = _build_nc()
    return _cache["nc"]


def _prep_in_maps(kern, bias, w_out_u, b_out_u, w_out_l, b_out_l):
    kaug = np.concatenate([kern, bias[None, :]], axis=0)  # [65, 512]
    ka = np.ascontiguousarray(
        kaug.T.reshape(KC, 128, 65).transpose(1, 0, 2)
    )  # [128, KC, 65]; ka[p,k,m] = kaug[m, k*128+p]

    in_maps = []
    for c in range(NCORES):
        s = slice(c * BL, (c + 1) * BL)
        in_maps.append(
            {
                "wu": w_out_u[s, 0].reshape(BL, KC, 128, N),
                "wl": w_out_l[s, 0].reshape(BL, KC, 128, N),
                "ka": ka,
                "bb": np.ascontiguousarray(
                    np.stack([b_out_u[s, 0], b_out_l[s, 0]], axis=1)
                ),
            }
        )
    return in_maps


def _assemble(results):
    ow = np.concatenate([r["ow"] for r in results], axis=0)  # [B, 2, 64, N]
    ob = np.concatenate([r["ob"] for r in results], axis=0)  # [B, 2, N]
    w_u = np.ascontiguousarray(ow[:, None, 0])  # [B, 1, 64, N]
    w_l = np.ascontiguousarray(ow[:, None, 1])
    b_u = np.ascontiguousarray(ob[:, None, 0])  # [B, 1, N]
    b_l = np.ascontiguousarray(ob[:, None, 1])
    return (w_u, b_u, w_l, b_l)


def kernel(**inputs):
    from concourse.bass_utils import run_bass_kernel_spmd

    kern = np.ascontiguousarray(np.asarray(inputs["kernel"], dtype=np.float32))
    bias = np.ascontiguousarray(np.asarray(inputs["bias"], dtype=np.float32))
    w_out_u = np.ascontiguousarray(np.asarray(inputs["w_out_u"], dtype=np.float32))
    b_out_u = np.ascontiguousarray(np.asarray(inputs["b_out_u"], dtype=np.float32))
    w_out_l = np.ascontiguousarray(np.asarray(inputs["w_out_l"], dtype=np.float32))
    b_out_l = np.ascontiguousarray(np.asarray(inputs["b_out_l"], dtype=np.float32))

    nc = _get_nc()
    in_maps = _prep_in_maps(kern, bias, w_out_u, b_out_u, w_out_l, b_out_l)
    res = run_bass_kernel_spmd(nc, in_maps, core_ids=list(range(NCORES)))
    return _assemble(res.results)
